# revision 18
# baseline (speedup 1.0000x reference)
"""DiagLinear (block-diagonal linear + output interleave + bias) on 8 TRN2 cores.

Reference computation (fp32):
    x:   (B=8, S=2048, P*DIN=4096)
    w:   (P=16, DOUT=256, DIN=256)
    b:   (4096,)
    y[b, s, o*P + p] = sum_i x[b, s, p*DIN + i] * w[p, o, i]  + bias[o*P+p]

Sharding: data parallel over the batch dim — core c computes batch c.

Per-core kernel (x_c: [2048, 4096] -> y_c: [2048, 4096]):
  for each 128-token tile, in 8 groups of 4 feature chunks:
    1. DMA x tile [128 tok, 4096 feat] (natural layout, prefetched)
    2. PE-transpose the group's 4 [128,128] chunks into PSUM (fp32r,
       1.5 cyc/row), ACT-copy to SBUF -> xT [128 feat, 512 tok-chunks]
    3. matmul (fp32r/TF32, 1 cyc/row at out-free 256):
         psum[tok, o] += xT_chunk.T @ w_chunk
    4. DVE adds bias and writes the (o,p)-interleaved output quarter
    5. DMA y tile [128, 4096] out

The transpose stream runs S=2 groups ahead of the matmul stream in the
same tile (not a full tile ahead), so stores trail loads by ~1 tile.
Weight is pre-laid-out on the host as lhs-ready [128, 8192] (i128 x
(p, c, o)); bias is pre-permuted to (p, o) order, DMA'd to one
partition, and replicated on-chip.
"""

import contextlib
import ctypes
import sys
import types
from collections import deque

import numpy as np

from concourse import bass, mybir, tile
from concourse.bass_utils import run_bass_kernel_spmd


def _install_ntff_shim():
    """Provide antenv.axon_hooks (missing in this image) so trace=True can
    capture NTFF profiles via the axon .so.  Only used when profiling."""
    if "antenv.axon_hooks" in sys.modules:
        return
    so = "/opt/axon/libaxon_pjrt.so"
    try:
        lib = ctypes.CDLL(so)
        lib.axon_start_nrt_profile.argtypes = [
            ctypes.POINTER(ctypes.c_int64),
            ctypes.c_size_t,
        ]
        lib.axon_start_nrt_profile.restype = ctypes.c_int64
        lib.axon_stop_nrt_profile.argtypes = [ctypes.c_char_p]
        lib.axon_stop_nrt_profile.restype = ctypes.c_int64
    except (OSError, AttributeError):
        return

    @contextlib.contextmanager
    def hook(output_dir, device_ids):
        import jax

        jax.devices()
        if device_ids:
            ids = (ctypes.c_int64 * len(device_ids))(*device_ids)
            rc = lib.axon_start_nrt_profile(ids, len(device_ids))
        else:
            rc = lib.axon_start_nrt_profile(None, 0)
        if rc != 0:
            raise RuntimeError(f"axon_start_nrt_profile rc={rc}")
        try:
            yield
        finally:
            n = lib.axon_stop_nrt_profile(str(output_dir).encode())
            print(f"ntff profile: {n} file(s) -> {output_dir}", file=sys.stderr)

    mod = types.ModuleType("antenv.axon_hooks")
    mod.get_axon_ntff_profile_hook = lambda: hook
    mod.set_axon_ntff_profile_hook = lambda h: None
    sys.modules["antenv.axon_hooks"] = mod

P = 16
DIN = 256
DOUT = 256
B = 8
S = 2048
D = P * DIN  # 4096
T_TILE = 128
N_TILES = S // T_TILE  # 16
N_CHUNKS = D // 128  # 32 feature chunks of 128
F32 = mybir.dt.float32
F32R = mybir.dt.float32r
STAG = 2  # transpose stream runs this many groups ahead of matmuls


def _split_multi_waits(nc, max_waits=1):
    """This container's walrus build accepts at most one sync-wait per
    instruction; Tile attaches several.  Move the surplus onto dedicated
    single-wait EventSemaphore instructions right before the instruction
    on the same engine (same semantics: the engine is serial)."""
    n_split = 0
    for f in nc.m.functions:
        for bb in f.blocks:
            new_insts = []
            for inst in bb.instructions:
                si = inst.sync_info
                if si is not None and si.on_wait and len(si.on_wait) > max_waits:
                    waits = list(si.on_wait)
                    extra, keep = waits[:-max_waits], waits[-max_waits:]
                    for k, w in enumerate(extra):
                        nop = mybir.InstEventSemaphore(
                            name=f"{inst.name}-wsplit-{k}",
                            engine=inst.engine,
                            sync_info=mybir.SyncInfo(on_wait=[w], on_update=[]),
                        )
                        nc.register_instruction(nop)
                        new_insts.append(nop)
                        n_split += 1
                    inst.sync_info = mybir.SyncInfo(
                        on_wait=keep, on_update=list(si.on_update or [])
                    )
                new_insts.append(inst)
            bb.instructions[:] = new_insts
    return n_split


def build_nc():
    nc = bass.Bass()
    x_d = nc.declare_dram_parameter("x", [S, D], F32R, isOutput=False)
    i_d = nc.declare_dram_parameter("ident", [128, 128], F32R, isOutput=False)
    w_d = nc.declare_dram_parameter("w", [128, N_CHUNKS * DOUT], F32R, isOutput=False)
    b_d = nc.declare_dram_parameter("bias_po", [1, D], F32R, isOutput=False)
    o_d = nc.declare_dram_parameter("ones_row", [1, 128], F32R, isOutput=False)
    y_d = nc.declare_dram_parameter("y", [S, D], F32, isOutput=True)

    with tile.TileContext(nc) as tc:
        with (
            tc.tile_pool(name="const", bufs=1) as const_pool,
            tc.tile_pool(name="x0p", bufs=8) as pool_x0,
            tc.tile_pool(name="x_nat", bufs=1) as pool_x,
            tc.tile_pool(name="xt", bufs=6) as pool_xt,
            tc.tile_pool(name="y_sb", bufs=3) as pool_y,
            tc.tile_pool(name="ps_t", bufs=2, space="PSUM") as pool_pst,
            tc.tile_pool(name="ps_y", bufs=3, space="PSUM") as pool_psy,
        ):
            ident = const_pool.tile([128, 128], F32R)
            nc.sync.dma_start(ident[:], i_d[:])

            # bias: one 16 KB partition-0 row, replicated on-chip via a
            # ones-row PE matmul (saves the 2 MiB host-replicated transfer)
            bias_1p = const_pool.tile([1, D], F32R)
            nc.scalar.dma_start(bias_1p[:], b_d[:])
            ones_row = const_pool.tile([1, 128], F32R)
            nc.scalar.dma_start(ones_row[:], o_d[:])
            bias_sb = const_pool.tile([128, D], F32)

            # tile 0's x arrives as 8 independent group tiles so the first
            # transposes unblock after ~256 KiB instead of 2 MiB
            x0_parts = []
            for g in range(8):
                x0g = pool_x0.tile([128, 4 * 128], F32R)
                nc.sync.dma_start(x0g[:], x_d[0:T_TILE, g * 512 : (g + 1) * 512])
                x0_parts.append(x0g)

            # weights as 4 chunk tiles in j order so early matmuls don't wait
            # for the whole transfer
            n_wch = 4
            wch_cols = N_CHUNKS * DOUT // n_wch  # 2048 = 8 j-chunks
            w_tiles = []
            for k in range(n_wch):
                wt_k = const_pool.tile([128, wch_cols], F32R, tag=f"wt{k}")
                nc.scalar.dma_start(
                    wt_k[:], w_d[:, k * wch_cols : (k + 1) * wch_cols]
                )
                w_tiles.append(wt_k)

            def w_ap(j):
                return w_tiles[j // 8][:, (j % 8) * DOUT : (j % 8 + 1) * DOUT]

            def issue_x_load(tt):
                x_nat = pool_x.tile([128, D], F32R, tag=f"x{tt % 4}")
                nc.sync.dma_start(
                    x_nat[:], x_d[tt * T_TILE : (tt + 1) * T_TILE, :]
                )
                return x_nat

            def emit_group_transpose(tt, g, x_src):
                """Transpose chunks 4g..4g+3 of tile tt into an xT tile."""
                ps_t = pool_pst.tile([128, 512], F32)
                for jj in range(4):
                    j = 4 * g + jj
                    src = (
                        x0_parts[g][:, jj * 128 : (jj + 1) * 128]
                        if tt == 0
                        else x_src[:, j * 128 : (j + 1) * 128]
                    )
                    nc.tensor.transpose(
                        ps_t[:, jj * 128 : (jj + 1) * 128].bitcast(F32R),
                        src,
                        ident[:],
                    )
                xt = pool_xt.tile([128, 512], F32R)
                nc.scalar.copy(xt[:], ps_t[:])
                return xt

            def emit_group_matmuls(g, xt, psy):
                """Matmuls for blocks 2g, 2g+1 (consume chunks 4g..4g+3)."""
                for pb in (0, 1):
                    p = 2 * g + pb
                    pp = p % 4
                    for c in (0, 1):
                        j = 2 * p + c
                        sl = slice((j % 4) * 128, (j % 4 + 1) * 128)
                        nc.tensor.matmul(
                            psy[:, pp * DOUT : (pp + 1) * DOUT],
                            xt[:, sl],
                            w_ap(j),
                            start=(c == 0),
                            stop=(c == 1),
                        )

            def emit_bias_bcast(q):
                # matmul out must stay within one PSUM bank (512 f32)
                ps_b = pool_psy.tile([128, 1024], F32, tag="psq")
                for h in (0, 1):
                    nc.tensor.matmul(
                        ps_b[:, 512 * h : 512 * (h + 1)],
                        ones_row[:],
                        bias_1p[:, 1024 * q + 512 * h : 1024 * q + 512 * (h + 1)],
                        start=True,
                        stop=True,
                    )
                nc.scalar.copy(bias_sb[:, 1024 * q : 1024 * (q + 1)], ps_b[:])

            # x prefetch: tiles 1 and 2 up front, then t+3 at tile-t start
            x_bufs = {}
            for tt in (1, 2):
                x_bufs[tt] = issue_x_load(tt)

            # The transpose stream runs a constant STAG groups ahead of the
            # matmul stream (a ramped lead was tried and regressed: the
            # pipeline flush is DVE-paced, so pre-transposing the tail buys
            # nothing while the transpose bursts add mid-run PE stalls)
            total_slots = N_TILES * 8

            def lead(s):
                return STAG

            t_entries = [
                (tau, gam) for tau in range(N_TILES) for gam in range(8)
            ]
            t_cursor = 0
            xt_fifo = deque()

            def emit_transposes_until(target):
                nonlocal t_cursor
                while t_cursor < min(total_slots, target):
                    tau, gam = t_entries[t_cursor]
                    xt_fifo.append(
                        emit_group_transpose(tau, gam, x_bufs.get(tau))
                    )
                    if gam == 7:
                        x_bufs.pop(tau, None)
                    t_cursor += 1

            # Bias quarters 0/1 broadcast first (needed by the first DVE
            # adds), 2/3 after the prologue transposes so they don't delay
            # the first xT copies
            emit_bias_bcast(0)
            emit_bias_bcast(1)
            emit_transposes_until(STAG)
            emit_bias_bcast(2)
            emit_bias_bcast(3)

            for t in range(N_TILES):
                if t + 3 <= N_TILES - 1:
                    x_bufs[t + 3] = issue_x_load(t + 3)
                y_sb = pool_y.tile([128, D], F32)
                psy = None
                for g in range(8):
                    s = t * 8 + g
                    emit_transposes_until(s + 1 + lead(s))
                    if g % 2 == 0:
                        psy = pool_psy.tile([128, 1024], F32, tag="psq")
                    emit_group_matmuls(g, xt_fifo.popleft(), psy)
                    if g % 2 == 1:
                        q = g // 2
                        # psum quarter in (pp, o); y cols j = 16o + 4q + pp
                        y_view = y_sb[:].rearrange("t (o p) -> t o p", p=P)
                        nc.vector.tensor_add(
                            y_view[:, :, 4 * q : 4 * q + 4],
                            psy[:].rearrange("t (p o) -> t o p", p=4),
                            bias_sb[:, 1024 * q : 1024 * (q + 1)].rearrange(
                                "t (p o) -> t o p", p=4
                            ),
                        )

                nc.scalar.dma_start(y_d[t * T_TILE : (t + 1) * T_TILE, :], y_sb[:])

    _split_multi_waits(nc)
    return nc


def _host_weight(weight):
    # w_host[i128, (2p + c)*DOUT + o] = weight[p, o, 128c + i128]
    wt = weight.transpose(0, 2, 1).reshape(P, 2, 128, DOUT)  # [p, c, i128, o]
    return np.ascontiguousarray(
        wt.transpose(2, 0, 1, 3).reshape(128, N_CHUNKS * DOUT)
    ).astype(np.float32)


def _host_bias(bias):
    # (p, o) order on a single partition row
    return np.ascontiguousarray(bias.reshape(DOUT, P).T).reshape(1, D).astype(
        np.float32
    )


def kernel(inputs, weight, bias, _trace=False):
    inputs = np.asarray(inputs, dtype=np.float32)
    weight = np.asarray(weight, dtype=np.float32)
    bias = np.asarray(bias, dtype=np.float32)
    assert inputs.shape == (B, S, D)

    if _trace:
        _install_ntff_shim()
    nc = build_nc()
    common = {
        "ident": np.eye(128, dtype=np.float32),
        "bias_po": _host_bias(bias),
        "ones_row": np.ones((1, 128), dtype=np.float32),
        "w": _host_weight(weight),
    }
    in_maps = [
        {"x": np.ascontiguousarray(inputs[c]), **common} for c in range(B)
    ]
    res = run_bass_kernel_spmd(nc, in_maps, core_ids=list(range(8)), trace=_trace)
    out = np.stack([res.results[c]["y"] for c in range(B)], axis=0)
    if _trace:
        kernel.last_exec_time_ns = res.exec_time_ns
        kernel.last_results = res
    return out


# revision 20
# speedup vs baseline: 1.0830x; 1.0830x over previous
"""DiagLinear (block-diagonal linear + output interleave + bias) on 8 TRN2 cores.

Reference computation (fp32):
    x:   (B=8, S=2048, P*DIN=4096)
    w:   (P=16, DOUT=256, DIN=256)
    b:   (4096,)
    y[b, s, o*P + p] = sum_i x[b, s, p*DIN + i] * w[p, o, i]  + bias[o*P+p]

Sharding: data parallel over the batch dim — core c computes batch c.

Per-core kernel (x_c: [2048, 4096] -> y_c: [2048, 4096]):
  for each 128-token tile, in 8 groups of 4 feature chunks:
    1. DMA x tile [128 tok, 4096 feat] (natural layout, prefetched)
    2. PE-transpose the group's 4 [128,128] chunks into PSUM (fp32r,
       1.5 cyc/row), ACT-copy to SBUF -> xT [128 feat, 512 tok-chunks]
    3. matmul (fp32r/TF32, 1 cyc/row at out-free 256):
         psum[tok, o] += xT_chunk.T @ w_chunk
    4. DVE adds bias and writes the (o,p)-interleaved output quarter
    5. DMA y tile [128, 4096] out

The transpose stream runs S=2 groups ahead of the matmul stream in the
same tile (not a full tile ahead), so stores trail loads by ~1 tile.
Weight is pre-laid-out on the host as lhs-ready [128, 8192] (i128 x
(p, c, o)); bias is pre-permuted to (p, o) order, DMA'd to one
partition, and replicated on-chip.
"""

import contextlib
import ctypes
import sys
import types
from collections import deque

import numpy as np

from concourse import bass, mybir, tile
from concourse.bass_utils import run_bass_kernel_spmd


def _install_ntff_shim():
    """Provide antenv.axon_hooks (missing in this image) so trace=True can
    capture NTFF profiles via the axon .so.  Only used when profiling."""
    if "antenv.axon_hooks" in sys.modules:
        return
    so = "/opt/axon/libaxon_pjrt.so"
    try:
        lib = ctypes.CDLL(so)
        lib.axon_start_nrt_profile.argtypes = [
            ctypes.POINTER(ctypes.c_int64),
            ctypes.c_size_t,
        ]
        lib.axon_start_nrt_profile.restype = ctypes.c_int64
        lib.axon_stop_nrt_profile.argtypes = [ctypes.c_char_p]
        lib.axon_stop_nrt_profile.restype = ctypes.c_int64
    except (OSError, AttributeError):
        return

    @contextlib.contextmanager
    def hook(output_dir, device_ids):
        import jax

        jax.devices()
        if device_ids:
            ids = (ctypes.c_int64 * len(device_ids))(*device_ids)
            rc = lib.axon_start_nrt_profile(ids, len(device_ids))
        else:
            rc = lib.axon_start_nrt_profile(None, 0)
        if rc != 0:
            raise RuntimeError(f"axon_start_nrt_profile rc={rc}")
        try:
            yield
        finally:
            n = lib.axon_stop_nrt_profile(str(output_dir).encode())
            print(f"ntff profile: {n} file(s) -> {output_dir}", file=sys.stderr)

    mod = types.ModuleType("antenv.axon_hooks")
    mod.get_axon_ntff_profile_hook = lambda: hook
    mod.set_axon_ntff_profile_hook = lambda h: None
    sys.modules["antenv.axon_hooks"] = mod

P = 16
DIN = 256
DOUT = 256
B = 8
S = 2048
D = P * DIN  # 4096
T_TILE = 128
N_TILES = S // T_TILE  # 16
N_CHUNKS = D // 128  # 32 feature chunks of 128
F32 = mybir.dt.float32
F32R = mybir.dt.float32r
STAG = 2  # transpose stream runs this many groups ahead of matmuls


def _split_multi_waits(nc, max_waits=1):
    """This container's walrus build accepts at most one sync-wait per
    instruction; Tile attaches several.  Move the surplus onto dedicated
    single-wait EventSemaphore instructions right before the instruction
    on the same engine (same semantics: the engine is serial)."""
    n_split = 0
    for f in nc.m.functions:
        for bb in f.blocks:
            new_insts = []
            for inst in bb.instructions:
                si = inst.sync_info
                if si is not None and si.on_wait and len(si.on_wait) > max_waits:
                    waits = list(si.on_wait)
                    extra, keep = waits[:-max_waits], waits[-max_waits:]
                    for k, w in enumerate(extra):
                        nop = mybir.InstEventSemaphore(
                            name=f"{inst.name}-wsplit-{k}",
                            engine=inst.engine,
                            sync_info=mybir.SyncInfo(on_wait=[w], on_update=[]),
                        )
                        nc.register_instruction(nop)
                        new_insts.append(nop)
                        n_split += 1
                    inst.sync_info = mybir.SyncInfo(
                        on_wait=keep, on_update=list(si.on_update or [])
                    )
                new_insts.append(inst)
            bb.instructions[:] = new_insts
    return n_split


def build_nc():
    nc = bass.Bass()
    x_d = nc.declare_dram_parameter("x", [S, D], F32R, isOutput=False)
    i_d = nc.declare_dram_parameter("ident", [128, 128], F32R, isOutput=False)
    w_d = nc.declare_dram_parameter("w", [128, N_CHUNKS * DOUT], F32R, isOutput=False)
    b_d = nc.declare_dram_parameter("bias_po", [1, D], F32R, isOutput=False)
    o_d = nc.declare_dram_parameter("ones_row", [1, 128], F32R, isOutput=False)
    y_d = nc.declare_dram_parameter("y", [S, D], F32, isOutput=True)

    with tile.TileContext(nc) as tc:
        with (
            tc.tile_pool(name="const", bufs=1) as const_pool,
            tc.tile_pool(name="x0p", bufs=8) as pool_x0,
            tc.tile_pool(name="x_nat", bufs=1) as pool_x,
            tc.tile_pool(name="xt", bufs=6) as pool_xt,
            tc.tile_pool(name="y_sb", bufs=2) as pool_y,
            tc.tile_pool(name="ps_t", bufs=2, space="PSUM") as pool_pst,
            tc.tile_pool(name="ps_y", bufs=3, space="PSUM") as pool_psy,
        ):
            ident = const_pool.tile([128, 128], F32R)
            nc.sync.dma_start(ident[:], i_d[:])

            # bias: one 16 KB partition-0 row, replicated on-chip via a
            # ones-row PE matmul (saves the 2 MiB host-replicated transfer)
            bias_1p = const_pool.tile([1, D], F32R)
            nc.scalar.dma_start(bias_1p[:], b_d[:])
            ones_row = const_pool.tile([1, 128], F32R)
            nc.scalar.dma_start(ones_row[:], o_d[:])
            bias_sb = const_pool.tile([128, D], F32)

            # tile 0's x arrives as 8 independent group tiles so the first
            # transposes unblock after ~256 KiB instead of 2 MiB
            x0_parts = []
            for g in range(8):
                x0g = pool_x0.tile([128, 4 * 128], F32R)
                nc.sync.dma_start(x0g[:], x_d[0:T_TILE, g * 512 : (g + 1) * 512])
                x0_parts.append(x0g)

            # weights as 4 chunk tiles in j order so early matmuls don't wait
            # for the whole transfer
            n_wch = 4
            wch_cols = N_CHUNKS * DOUT // n_wch  # 2048 = 8 j-chunks
            w_tiles = []
            for k in range(n_wch):
                wt_k = const_pool.tile([128, wch_cols], F32R, tag=f"wt{k}")
                nc.scalar.dma_start(
                    wt_k[:], w_d[:, k * wch_cols : (k + 1) * wch_cols]
                )
                w_tiles.append(wt_k)

            def w_ap(j):
                return w_tiles[j // 8][:, (j % 8) * DOUT : (j % 8 + 1) * DOUT]

            def issue_x_load(tt):
                x_nat = pool_x.tile([128, D], F32R, tag=f"x{tt % 4}")
                nc.sync.dma_start(
                    x_nat[:], x_d[tt * T_TILE : (tt + 1) * T_TILE, :]
                )
                return x_nat

            def emit_group_transpose(tt, g, x_src):
                """Transpose chunks 4g..4g+3 of tile tt into an xT tile."""
                ps_t = pool_pst.tile([128, 512], F32)
                for jj in range(4):
                    j = 4 * g + jj
                    src = (
                        x0_parts[g][:, jj * 128 : (jj + 1) * 128]
                        if tt == 0
                        else x_src[:, j * 128 : (j + 1) * 128]
                    )
                    nc.tensor.transpose(
                        ps_t[:, jj * 128 : (jj + 1) * 128].bitcast(F32R),
                        src,
                        ident[:],
                    )
                xt = pool_xt.tile([128, 512], F32R)
                nc.scalar.copy(xt[:], ps_t[:])
                return xt

            def emit_group_matmuls(g, xt, psy):
                """Matmuls for blocks 2g, 2g+1 (consume chunks 4g..4g+3)."""
                for pb in (0, 1):
                    p = 2 * g + pb
                    pp = p % 4
                    for c in (0, 1):
                        j = 2 * p + c
                        sl = slice((j % 4) * 128, (j % 4 + 1) * 128)
                        nc.tensor.matmul(
                            psy[:, pp * DOUT : (pp + 1) * DOUT],
                            xt[:, sl],
                            w_ap(j),
                            start=(c == 0),
                            stop=(c == 1),
                        )

            def emit_bias_bcast(q):
                # matmul out must stay within one PSUM bank (512 f32)
                ps_b = pool_psy.tile([128, 1024], F32, tag="psq")
                for h in (0, 1):
                    nc.tensor.matmul(
                        ps_b[:, 512 * h : 512 * (h + 1)],
                        ones_row[:],
                        bias_1p[:, 1024 * q + 512 * h : 1024 * q + 512 * (h + 1)],
                        start=True,
                        stop=True,
                    )
                nc.scalar.copy(bias_sb[:, 1024 * q : 1024 * (q + 1)], ps_b[:])

            # x prefetch: tiles 1 and 2 up front, then t+3 at tile-t start
            x_bufs = {}
            for tt in (1, 2):
                x_bufs[tt] = issue_x_load(tt)

            # The transpose stream runs a constant STAG groups ahead of the
            # matmul stream (a ramped lead was tried and regressed: the
            # pipeline flush is DVE-paced, so pre-transposing the tail buys
            # nothing while the transpose bursts add mid-run PE stalls)
            total_slots = N_TILES * 8

            def lead(s):
                return STAG

            t_entries = [
                (tau, gam) for tau in range(N_TILES) for gam in range(8)
            ]
            t_cursor = 0
            xt_fifo = deque()

            def emit_transposes_until(target):
                nonlocal t_cursor
                while t_cursor < min(total_slots, target):
                    tau, gam = t_entries[t_cursor]
                    xt_fifo.append(
                        emit_group_transpose(tau, gam, x_bufs.get(tau))
                    )
                    if gam == 7:
                        x_bufs.pop(tau, None)
                    t_cursor += 1

            # Bias quarters 0/1 broadcast first (needed by the first DVE
            # adds), 2/3 after the prologue transposes so they don't delay
            # the first xT copies
            emit_bias_bcast(0)
            emit_bias_bcast(1)
            emit_transposes_until(STAG)
            emit_bias_bcast(2)
            emit_bias_bcast(3)

            for t in range(N_TILES):
                if t + 3 <= N_TILES - 1:
                    x_bufs[t + 3] = issue_x_load(t + 3)
                y_sb = pool_y.tile([128, D], F32)
                psy = None
                for g in range(8):
                    s = t * 8 + g
                    emit_transposes_until(s + 1 + lead(s))
                    if g % 2 == 0:
                        psy = pool_psy.tile([128, 1024], F32, tag="psq")
                    emit_group_matmuls(g, xt_fifo.popleft(), psy)
                    if g % 2 == 1:
                        q = g // 2
                        # psum quarter in (pp, o); y cols j = 16o + 4q + pp
                        y_view = y_sb[:].rearrange("t (o p) -> t o p", p=P)
                        nc.vector.tensor_add(
                            y_view[:, :, 4 * q : 4 * q + 4],
                            psy[:].rearrange("t (p o) -> t o p", p=4),
                            bias_sb[:, 1024 * q : 1024 * (q + 1)].rearrange(
                                "t (p o) -> t o p", p=4
                            ),
                        )

                # stores ride the SAME ring as the x loads: a single HWDGE
                # FIFO runs one transfer at a time, so loads and stores never
                # interleave at packet granularity (HBM R/W turnaround costs
                # ~20% when they do)
                nc.sync.dma_start(y_d[t * T_TILE : (t + 1) * T_TILE, :], y_sb[:])

    _split_multi_waits(nc)
    return nc


def _host_weight(weight):
    # w_host[i128, (2p + c)*DOUT + o] = weight[p, o, 128c + i128]
    wt = weight.transpose(0, 2, 1).reshape(P, 2, 128, DOUT)  # [p, c, i128, o]
    return np.ascontiguousarray(
        wt.transpose(2, 0, 1, 3).reshape(128, N_CHUNKS * DOUT)
    ).astype(np.float32)


def _host_bias(bias):
    # (p, o) order on a single partition row
    return np.ascontiguousarray(bias.reshape(DOUT, P).T).reshape(1, D).astype(
        np.float32
    )


def kernel(inputs, weight, bias, _trace=False):
    inputs = np.asarray(inputs, dtype=np.float32)
    weight = np.asarray(weight, dtype=np.float32)
    bias = np.asarray(bias, dtype=np.float32)
    assert inputs.shape == (B, S, D)

    if _trace:
        _install_ntff_shim()
    nc = build_nc()
    common = {
        "ident": np.eye(128, dtype=np.float32),
        "bias_po": _host_bias(bias),
        "ones_row": np.ones((1, 128), dtype=np.float32),
        "w": _host_weight(weight),
    }
    in_maps = [
        {"x": np.ascontiguousarray(inputs[c]), **common} for c in range(B)
    ]
    res = run_bass_kernel_spmd(nc, in_maps, core_ids=list(range(8)), trace=_trace)
    out = np.stack([res.results[c]["y"] for c in range(B)], axis=0)
    if _trace:
        kernel.last_exec_time_ns = res.exec_time_ns
        kernel.last_results = res
    return out


# revision 21
# speedup vs baseline: 1.1020x; 1.0175x over previous
"""DiagLinear (block-diagonal linear + output interleave + bias) on 8 TRN2 cores.

Reference computation (fp32):
    x:   (B=8, S=2048, P*DIN=4096)
    w:   (P=16, DOUT=256, DIN=256)
    b:   (4096,)
    y[b, s, o*P + p] = sum_i x[b, s, p*DIN + i] * w[p, o, i]  + bias[o*P+p]

Sharding: data parallel over the batch dim — core c computes batch c.

Per-core kernel (x_c: [2048, 4096] -> y_c: [2048, 4096]):
  for each 128-token tile, in 8 groups of 4 feature chunks:
    1. DMA x tile [128 tok, 4096 feat] (natural layout, prefetched)
    2. PE-transpose the group's 4 [128,128] chunks into PSUM (fp32r,
       1.5 cyc/row), ACT-copy to SBUF -> xT [128 feat, 512 tok-chunks]
    3. matmul (fp32r/TF32, 1 cyc/row at out-free 256):
         psum[tok, o] += xT_chunk.T @ w_chunk
    4. DVE adds bias and writes the (o,p)-interleaved output quarter
    5. DMA y tile [128, 4096] out

The transpose stream runs S=2 groups ahead of the matmul stream in the
same tile (not a full tile ahead), so stores trail loads by ~1 tile.
Weight is pre-laid-out on the host as lhs-ready [128, 8192] (i128 x
(p, c, o)); bias is pre-permuted to (p, o) order, DMA'd to one
partition, and replicated on-chip.
"""

import contextlib
import ctypes
import sys
import types
from collections import deque

import numpy as np

from concourse import bass, mybir, tile
from concourse.bass_utils import run_bass_kernel_spmd


def _install_ntff_shim():
    """Provide antenv.axon_hooks (missing in this image) so trace=True can
    capture NTFF profiles via the axon .so.  Only used when profiling."""
    if "antenv.axon_hooks" in sys.modules:
        return
    so = "/opt/axon/libaxon_pjrt.so"
    try:
        lib = ctypes.CDLL(so)
        lib.axon_start_nrt_profile.argtypes = [
            ctypes.POINTER(ctypes.c_int64),
            ctypes.c_size_t,
        ]
        lib.axon_start_nrt_profile.restype = ctypes.c_int64
        lib.axon_stop_nrt_profile.argtypes = [ctypes.c_char_p]
        lib.axon_stop_nrt_profile.restype = ctypes.c_int64
    except (OSError, AttributeError):
        return

    @contextlib.contextmanager
    def hook(output_dir, device_ids):
        import jax

        jax.devices()
        if device_ids:
            ids = (ctypes.c_int64 * len(device_ids))(*device_ids)
            rc = lib.axon_start_nrt_profile(ids, len(device_ids))
        else:
            rc = lib.axon_start_nrt_profile(None, 0)
        if rc != 0:
            raise RuntimeError(f"axon_start_nrt_profile rc={rc}")
        try:
            yield
        finally:
            n = lib.axon_stop_nrt_profile(str(output_dir).encode())
            print(f"ntff profile: {n} file(s) -> {output_dir}", file=sys.stderr)

    mod = types.ModuleType("antenv.axon_hooks")
    mod.get_axon_ntff_profile_hook = lambda: hook
    mod.set_axon_ntff_profile_hook = lambda h: None
    sys.modules["antenv.axon_hooks"] = mod

P = 16
DIN = 256
DOUT = 256
B = 8
S = 2048
D = P * DIN  # 4096
T_TILE = 128
N_TILES = S // T_TILE  # 16
N_CHUNKS = D // 128  # 32 feature chunks of 128
F32 = mybir.dt.float32
F32R = mybir.dt.float32r
STAG = 2  # transpose stream runs this many groups ahead of matmuls


def _split_multi_waits(nc, max_waits=1):
    """This container's walrus build accepts at most one sync-wait per
    instruction; Tile attaches several.  Move the surplus onto dedicated
    single-wait EventSemaphore instructions right before the instruction
    on the same engine (same semantics: the engine is serial)."""
    n_split = 0
    for f in nc.m.functions:
        for bb in f.blocks:
            new_insts = []
            for inst in bb.instructions:
                si = inst.sync_info
                if si is not None and si.on_wait and len(si.on_wait) > max_waits:
                    waits = list(si.on_wait)
                    extra, keep = waits[:-max_waits], waits[-max_waits:]
                    for k, w in enumerate(extra):
                        nop = mybir.InstEventSemaphore(
                            name=f"{inst.name}-wsplit-{k}",
                            engine=inst.engine,
                            sync_info=mybir.SyncInfo(on_wait=[w], on_update=[]),
                        )
                        nc.register_instruction(nop)
                        new_insts.append(nop)
                        n_split += 1
                    inst.sync_info = mybir.SyncInfo(
                        on_wait=keep, on_update=list(si.on_update or [])
                    )
                new_insts.append(inst)
            bb.instructions[:] = new_insts
    return n_split


def build_nc():
    nc = bass.Bass()
    x_d = nc.declare_dram_parameter("x", [S, D], F32R, isOutput=False)
    i_d = nc.declare_dram_parameter("ident", [128, 128], F32R, isOutput=False)
    w_d = nc.declare_dram_parameter("w", [128, N_CHUNKS * DOUT], F32R, isOutput=False)
    b_d = nc.declare_dram_parameter("bias_po", [1, D], F32R, isOutput=False)
    o_d = nc.declare_dram_parameter("ones_row", [1, 128], F32R, isOutput=False)
    y_d = nc.declare_dram_parameter("y", [S, D], F32, isOutput=True)

    with tile.TileContext(nc) as tc:
        with (
            tc.tile_pool(name="const", bufs=1) as const_pool,
            tc.tile_pool(name="x0p", bufs=8) as pool_x0,
            tc.tile_pool(name="x_nat", bufs=1) as pool_x,
            tc.tile_pool(name="xt", bufs=6) as pool_xt,
            tc.tile_pool(name="y_sb", bufs=2) as pool_y,
            tc.tile_pool(name="ps_t", bufs=2, space="PSUM") as pool_pst,
            tc.tile_pool(name="ps_y", bufs=3, space="PSUM") as pool_psy,
        ):
            ident = const_pool.tile([128, 128], F32R)
            nc.scalar.dma_start(ident[:], i_d[:])

            # bias: one 16 KB partition-0 row, replicated on-chip via a
            # ones-row PE matmul (saves the 2 MiB host-replicated transfer)
            bias_1p = const_pool.tile([1, D], F32R)
            nc.sync.dma_start(bias_1p[:], b_d[:])
            ones_row = const_pool.tile([1, 128], F32R)
            nc.sync.dma_start(ones_row[:], o_d[:])
            bias_sb = const_pool.tile([128, D], F32)

            # tile 0's x arrives as 8 independent group tiles so the first
            # transposes unblock after ~256 KiB instead of 2 MiB
            x0_parts = []
            for g in range(8):
                x0g = pool_x0.tile([128, 4 * 128], F32R)
                nc.scalar.dma_start(x0g[:], x_d[0:T_TILE, g * 512 : (g + 1) * 512])
                x0_parts.append(x0g)

            # weights as 4 chunk tiles in j order so early matmuls don't wait
            # for the whole transfer
            n_wch = 4
            wch_cols = N_CHUNKS * DOUT // n_wch  # 2048 = 8 j-chunks
            w_tiles = []
            for k in range(n_wch):
                wt_k = const_pool.tile([128, wch_cols], F32R, tag=f"wt{k}")
                nc.sync.dma_start(
                    wt_k[:], w_d[:, k * wch_cols : (k + 1) * wch_cols]
                )
                w_tiles.append(wt_k)

            def w_ap(j):
                return w_tiles[j // 8][:, (j % 8) * DOUT : (j % 8 + 1) * DOUT]

            def issue_x_load(tt):
                x_nat = pool_x.tile([128, D], F32R, tag=f"x{tt % 4}")
                nc.scalar.dma_start(
                    x_nat[:], x_d[tt * T_TILE : (tt + 1) * T_TILE, :]
                )
                return x_nat

            def emit_group_transpose(tt, g, x_src):
                """Transpose chunks 4g..4g+3 of tile tt into an xT tile."""
                ps_t = pool_pst.tile([128, 512], F32)
                for jj in range(4):
                    j = 4 * g + jj
                    src = (
                        x0_parts[g][:, jj * 128 : (jj + 1) * 128]
                        if tt == 0
                        else x_src[:, j * 128 : (j + 1) * 128]
                    )
                    nc.tensor.transpose(
                        ps_t[:, jj * 128 : (jj + 1) * 128].bitcast(F32R),
                        src,
                        ident[:],
                    )
                xt = pool_xt.tile([128, 512], F32R)
                nc.scalar.copy(xt[:], ps_t[:])
                return xt

            def emit_group_matmuls(g, xt, psy):
                """Matmuls for blocks 2g, 2g+1 (consume chunks 4g..4g+3)."""
                for pb in (0, 1):
                    p = 2 * g + pb
                    pp = p % 4
                    for c in (0, 1):
                        j = 2 * p + c
                        sl = slice((j % 4) * 128, (j % 4 + 1) * 128)
                        nc.tensor.matmul(
                            psy[:, pp * DOUT : (pp + 1) * DOUT],
                            xt[:, sl],
                            w_ap(j),
                            start=(c == 0),
                            stop=(c == 1),
                        )

            def emit_bias_bcast(q):
                # matmul out must stay within one PSUM bank (512 f32)
                ps_b = pool_psy.tile([128, 1024], F32, tag="psq")
                for h in (0, 1):
                    nc.tensor.matmul(
                        ps_b[:, 512 * h : 512 * (h + 1)],
                        ones_row[:],
                        bias_1p[:, 1024 * q + 512 * h : 1024 * q + 512 * (h + 1)],
                        start=True,
                        stop=True,
                    )
                nc.scalar.copy(bias_sb[:, 1024 * q : 1024 * (q + 1)], ps_b[:])

            # x prefetch: tiles 1 and 2 up front, then t+3 at tile-t start
            x_bufs = {}
            for tt in (1, 2):
                x_bufs[tt] = issue_x_load(tt)

            # The transpose stream runs a constant STAG groups ahead of the
            # matmul stream (a ramped lead was tried and regressed: the
            # pipeline flush is DVE-paced, so pre-transposing the tail buys
            # nothing while the transpose bursts add mid-run PE stalls)
            total_slots = N_TILES * 8

            def lead(s):
                return STAG

            t_entries = [
                (tau, gam) for tau in range(N_TILES) for gam in range(8)
            ]
            t_cursor = 0
            xt_fifo = deque()

            def emit_transposes_until(target):
                nonlocal t_cursor
                while t_cursor < min(total_slots, target):
                    tau, gam = t_entries[t_cursor]
                    xt_fifo.append(
                        emit_group_transpose(tau, gam, x_bufs.get(tau))
                    )
                    if gam == 7:
                        x_bufs.pop(tau, None)
                    t_cursor += 1

            # Bias quarters 0/1 broadcast first (needed by the first DVE
            # adds), 2/3 after the prologue transposes so they don't delay
            # the first xT copies
            emit_bias_bcast(0)
            emit_bias_bcast(1)
            emit_transposes_until(STAG)
            emit_bias_bcast(2)
            emit_bias_bcast(3)

            for t in range(N_TILES):
                if t + 3 <= N_TILES - 1:
                    x_bufs[t + 3] = issue_x_load(t + 3)
                y_sb = pool_y.tile([128, D], F32)
                psy = None
                for g in range(8):
                    s = t * 8 + g
                    emit_transposes_until(s + 1 + lead(s))
                    if g % 2 == 0:
                        psy = pool_psy.tile([128, 1024], F32, tag="psq")
                    emit_group_matmuls(g, xt_fifo.popleft(), psy)
                    if g % 2 == 1:
                        q = g // 2
                        # psum quarter in (pp, o); y cols j = 16o + 4q + pp
                        y_view = y_sb[:].rearrange("t (o p) -> t o p", p=P)
                        nc.vector.tensor_add(
                            y_view[:, :, 4 * q : 4 * q + 4],
                            psy[:].rearrange("t (p o) -> t o p", p=4),
                            bias_sb[:, 1024 * q : 1024 * (q + 1)].rearrange(
                                "t (p o) -> t o p", p=4
                            ),
                        )

                # stores get their own ring (sync) that does nothing
                # else: a store waiting on DVE can only delay later stores.
                # Loads live on the ACT ring, whose dma_starts never wait,
                # so they can't block the xT copies either.
                nc.sync.dma_start(y_d[t * T_TILE : (t + 1) * T_TILE, :], y_sb[:])

    _split_multi_waits(nc)
    return nc


def _host_weight(weight):
    # w_host[i128, (2p + c)*DOUT + o] = weight[p, o, 128c + i128]
    wt = weight.transpose(0, 2, 1).reshape(P, 2, 128, DOUT)  # [p, c, i128, o]
    return np.ascontiguousarray(
        wt.transpose(2, 0, 1, 3).reshape(128, N_CHUNKS * DOUT)
    ).astype(np.float32)


def _host_bias(bias):
    # (p, o) order on a single partition row
    return np.ascontiguousarray(bias.reshape(DOUT, P).T).reshape(1, D).astype(
        np.float32
    )


def kernel(inputs, weight, bias, _trace=False):
    inputs = np.asarray(inputs, dtype=np.float32)
    weight = np.asarray(weight, dtype=np.float32)
    bias = np.asarray(bias, dtype=np.float32)
    assert inputs.shape == (B, S, D)

    if _trace:
        _install_ntff_shim()
    nc = build_nc()
    common = {
        "ident": np.eye(128, dtype=np.float32),
        "bias_po": _host_bias(bias),
        "ones_row": np.ones((1, 128), dtype=np.float32),
        "w": _host_weight(weight),
    }
    in_maps = [
        {"x": np.ascontiguousarray(inputs[c]), **common} for c in range(B)
    ]
    res = run_bass_kernel_spmd(nc, in_maps, core_ids=list(range(8)), trace=_trace)
    out = np.stack([res.results[c]["y"] for c in range(B)], axis=0)
    if _trace:
        kernel.last_exec_time_ns = res.exec_time_ns
        kernel.last_results = res
    return out
